# revision 18
# baseline (speedup 1.0000x reference)
"""Causal self-attention (B=4, L=2048, C=1024, H=16) on 8 trn2 NeuronCores.

Sharding: hybrid (batch x head) tensor-parallel. Core c handles batch
b = c // 2 and heads [ (c%2)*8, (c%2)*8 + 8 ).  Each core computes a
partial projection output (L, C) for its batch; the host sums the two
partials per batch (the Wp row-sharded all-reduce done host-side).

All matmul inputs are bf16 (PSUM accumulation fp32).  The kernel is a
single software-pipelined PE stream structured so the Tensor engine
never starves (HAM stays at K=8/8):

  - wave0: qT/kT ch0 chains consume x-tiles as their DMAs land
  - attention groups (band it, head-pair p) run ST -> exp -> y with a
    jt-lag; qk waves ch1-3, v chains, projection bands and the softmax
    denominator broadcasts are emitted as independent PE filler so the
    scheduler can hide the exp (ACT) latency under them
  - diagonal score tiles are trimmed: ST/y matmuls only cover the
    causally valid i-range, exp is split for d>=2
  - projection of band b is emitted right after band b's normalization
Per-core dataflow details:
  qT,kT = Wq_h @ x.T      (d, L) layout, head-pair packed (128 rows)
  v_aug = x @ Wv_h.T      (L, d) + ones column for softmax denominators
  ST    = kT_j.T @ qT     two heads at base partitions 0/64 run in
                          separate PE row groups concurrently
  P     = exp(ST/8)       ACT, bf16 out; causal mask via DVE on the
                          128-wide diagonal sub-block only
  yT    = sum_j v_aug_j.T @ P   rows 0..63 out, row 64 = denominator
  yT   *= bcast(1/denom)  K=1 PE broadcast + DVE fast reciprocal
  out   = yT.T @ WpT      partial projection, DMA to DRAM (fp32)
"""
import sys
import os

sys.path.insert(0, "/opt/trn_rl_repo")

import numpy as np

B, L, C, H, HD = 4, 2048, 1024, 16, 64
NCORE = 8

_compiled = {}


def _build():
    import concourse.bass as bass
    import concourse.mybir as mybir
    import concourse.tile as tile
    from concourse import bacc

    dt = mybir.dt
    f32 = dt.float32
    bf16 = dt.bfloat16
    Exp = mybir.ActivationFunctionType.Exp
    PSUM = bass.MemorySpace.PSUM

    nc = bacc.Bacc("TRN2", target_bir_lowering=False, debug=False, num_devices=NCORE)

    xT8 = nc.dram_tensor("xT8", [8, 128, L], bf16, kind="ExternalInput")
    wqk8 = nc.dram_tensor("wqk8", [8, 128, 1024], bf16, kind="ExternalInput")
    wv8 = nc.dram_tensor("wv8", [8, 128, 512], bf16, kind="ExternalInput")
    wp4 = nc.dram_tensor("wp4", [4, 128, 1024], bf16, kind="ExternalInput")
    cm2 = nc.dram_tensor("cm2", [128, 256], bf16, kind="ExternalInput")
    pdb = nc.dram_tensor("pdb", [128, 16], f32, kind="ExternalInput")
    one64 = nc.dram_tensor("one64", [1, 64], bf16, kind="ExternalInput")
    one8 = nc.dram_tensor("one8", [128, 8], bf16, kind="ExternalInput")
    out = nc.dram_tensor("out", [L, C], f32, kind="ExternalOutput")

    with tile.TileContext(nc) as tc:
        with (
            tc.tile_pool(name="persist", bufs=1) as persist,
            tc.tile_pool(name="xw", bufs=1) as xw,
            tc.tile_pool(name="qkT", bufs=8) as qkp,
            tc.tile_pool(name="vaug", bufs=16) as vaugp,
            tc.tile_pool(name="yT", bufs=4) as ytp,
            tc.tile_pool(name="P", bufs=8) as Pp,
            tc.tile_pool(name="sd", bufs=4) as sdp,
            tc.tile_pool(name="rr", bufs=4) as rrp,
            tc.tile_pool(name="osb", bufs=2) as osbp,
        ):
            # small constants first (cheap DMAs)
            pb = persist.tile([128, 16], f32, tag="pb")
            nc.sync.dma_start(pb[:], pdb[:])
            ones = persist.tile([1, 64], bf16, tag="ones")
            nc.sync.dma_start(ones[:], one64[:])
            ones8 = persist.tile([128, 8], bf16, tag="ones8")
            nc.sync.dma_start(ones8[:], one8[:])
            cm = persist.tile([128, 256], bf16, tag="cm")
            nc.sync.dma_start(cm[:], cm2[:])

            # weights + x interleaved so wave0 can start on the first tile
            # input load split across the two HWDGE queues (sync + scalar)
            # so the transfers run in parallel and wave0 starts sooner
            wqkts, xts = [], []
            for kt in range(8):
                w = xw.tile([128, 1024], bf16, tag="wqk", name=f"wqk{kt}", bufs=8)
                (nc.sync if kt % 2 else nc.scalar).dma_start(w[:], wqk8[kt])
                wqkts.append(w)
                t = xw.tile([128, L], bf16, tag="x", name=f"x{kt}", bufs=8)
                # wave0 only reads cols 0:512 — land that head first so the
                # first chains start right after the runtime preamble
                (nc.scalar if kt % 2 else nc.sync).dma_start(
                    t[:, 0:512], xT8[kt][:, 0:512]
                )
                xts.append(t)
            for kt in range(8):
                (nc.scalar if kt % 2 else nc.sync).dma_start(
                    xts[kt][:, 512:L], xT8[kt][:, 512:L]
                )
            wvts = []
            for kt in range(8):
                t = xw.tile([128, 512], bf16, tag="wv", name=f"wv{kt}", bufs=8)
                (nc.scalar if kt % 2 else nc.sync).dma_start(t[:], wv8[kt])
                wvts.append(t)
            wpts = []
            for kt4 in range(4):
                t = xw.tile([128, 1024], bf16, tag="wp", name=f"wp{kt4}", bufs=4)
                (nc.scalar if kt4 % 2 else nc.sync).dma_start(t[:], wp4[kt4])
                wpts.append(t)

            # warm the exp activation table while wave0 runs
            scratch = persist.tile([128, 16], f32, tag="scr")
            nc.scalar.activation(scratch[:], pb[:], Exp, scale=0.0)

            qts = [qkp.tile([128, L], bf16, tag="qk", name=f"q{p}") for p in range(4)]
            kts = [qkp.tile([128, L], bf16, tag="qk", name=f"k{p}") for p in range(4)]
            vats = [
                vaugp.tile([128, 8, 65], bf16, tag="va", name=f"va{it}")
                for it in range(16)
            ]
            yts = [ytp.tile([128, L], bf16, tag="yt", name=f"yt{p}") for p in range(4)]

            # ---- wave0: qk chains for ch 0 (8 chains, one PSUM bank each);
            # kt-outer issue so chains advance as x tiles arrive ----
            with tc.tile_pool(name="mmA", bufs=8, space=PSUM) as mmA:
                ps0 = [
                    mmA.tile([128, 512], f32, tag="mm", name=f"w0_{c8}")
                    for c8 in range(8)
                ]
                for kt in range(8):
                    for c8 in range(8):
                        nc.tensor.matmul(
                            ps0[c8][:],
                            wqkts[kt][:, c8 * 128 : (c8 + 1) * 128],
                            xts[kt][:, 0:512],
                            start=(kt == 0),
                            stop=(kt == 7),
                        )
                for c8 in range(8):
                    p, w = c8 // 2, c8 % 2
                    dst = qts[p] if w == 0 else kts[p]
                    nc.vector.tensor_copy(dst[:, 0:512], ps0[c8][:])

            # ---- attention + filler stream ----
            with (
                tc.tile_pool(name="stps", bufs=2, space=PSUM) as stps,
                tc.tile_pool(name="fillps", bufs=2, space=PSUM) as fillps,
                tc.tile_pool(name="yps", bufs=2, space=PSUM) as yps,
            ):
                def emit_qk_chain(p, w, ch):
                    ps = fillps.tile([128, 512], f32, tag="fill", name="qkc")
                    c8 = 2 * p + w
                    for kt in range(8):
                        nc.tensor.matmul(
                            ps[:],
                            wqkts[kt][:, c8 * 128 : (c8 + 1) * 128],
                            xts[kt][:, ch * 512 : (ch + 1) * 512],
                            start=(kt == 0),
                            stop=(kt == 7),
                        )
                    dst = qts[p] if w == 0 else kts[p]
                    nc.vector.tensor_copy(
                        dst[:, ch * 512 : (ch + 1) * 512], ps[:]
                    )

                def emit_v_chain(it):
                    ps = fillps.tile([128, 512], f32, tag="fill", name="vc")
                    for kt in range(8):
                        nc.tensor.matmul(
                            ps[:],
                            xts[kt][:, it * 128 : (it + 1) * 128],
                            wvts[kt][:],
                            start=(kt == 0),
                            stop=(kt == 7),
                        )
                    va = vats[it]
                    nc.vector.tensor_scalar_mul(
                        va[:, :, 0:64],
                        ps[:].rearrange("p (h e) -> p h e", e=64),
                        pb[:, it : it + 1],
                    )
                    nc.vector.tensor_scalar_mul(
                        va[:, :, 64:65],
                        ones8[:].rearrange("p (h o) -> p h o", o=1),
                        pb[:, it : it + 1],
                    )

                def emit_proj_ic(ic):
                    osb = osbp.tile([128, 1024], f32, tag="o", name="osb")
                    for ch in range(2):
                        ps = fillps.tile([128, 512], f32, tag="fill", name="pj")
                        for kt4 in range(4):
                            nc.tensor.matmul(
                                ps[:],
                                yts[kt4][:, ic * 128 : (ic + 1) * 128],
                                wpts[kt4][:, ch * 512 : (ch + 1) * 512],
                                start=(kt4 == 0),
                                stop=(kt4 == 3),
                            )
                        nc.vector.tensor_copy(
                            osb[:, ch * 512 : (ch + 1) * 512], ps[:]
                        )
                    nc.sync.dma_start(out[ic * 128 : (ic + 1) * 128, :], osb[:])

                # filler queue: list of closures, drained between attention
                # tiles; order = priority hint for the scheduler
                from collections import deque

                filler = deque()
                for ch in range(1, 4):
                    for c8 in range(8):
                        filler.append((f"w{ch}", lambda p=c8 // 2, w=c8 % 2,
                                       c=ch: emit_qk_chain(p, w, c)))
                for it in range(16):
                    filler.append((f"v{it}", lambda i=it: emit_v_chain(i)))

                def drain_required(b):
                    """Everything band b's STs/y's need must be emitted."""
                    need_w = {f"w{c}" for c in range(1, b + 1)}
                    need_v = {f"v{i}" for i in range(4 * b + 4)}
                    need = need_w | need_v
                    keep = deque()
                    while filler:
                        tag, fn = filler.popleft()
                        if tag in need:
                            fn()
                        else:
                            keep.append((tag, fn))
                    filler.extend(keep)

                def emit_filler(n):
                    for _ in range(n):
                        if not filler:
                            return
                        _, fn = filler.popleft()
                        fn()

                LAG = 6

                # bands 2 and 3 are interleaved group-wise: band 3 is
                # ACT-heavy (longest exp rows) while band 2 still has qk
                # wave 3 + v chains as PE filler — mixing them keeps both
                # engines fed through the back half of the kernel
                order = [(0, 0), (0, 1), (0, 2), (0, 3),
                         (1, 0), (1, 1), (1, 2), (1, 3),
                         (2, 0), (3, 0), (2, 1), (3, 1),
                         (2, 2), (3, 2), (2, 3), (3, 3)]
                def emit_st_g(b, p, jt, Ps, ptag="P", pbufs=None):
                    d = jt - 4 * b  # diagonal alignment, >=0 on diag
                    off = 128 * d if d > 0 else 0
                    qT, kT = qts[p], kts[p]
                    stp = stps.tile([128, 1024], f32, tag="st", name="stp")
                    for e in (0, 1):
                        ro = e * 64
                        nc.tensor.matmul(
                            stp[:, e * 512 + off : (e + 1) * 512],
                            kT[ro : ro + 64, jt * 128 : (jt + 1) * 128],
                            qT[ro : ro + 64, b * 512 + off : (b + 1) * 512],
                            start=True,
                            stop=True,
                        )
                    P = Pp.tile([128, 1024], bf16, tag=ptag, name="P",
                                bufs=pbufs)
                    if d >= 1:
                        # exp only the causally valid column range of
                        # both heads in one strided instruction (also
                        # avoids reading never-written PSUM)
                        sv = stp[:].rearrange(
                            "p (t i) -> p t i", t=2
                        )[:, :, off:512]
                        pv3 = P[:].rearrange(
                            "p (t i) -> p t i", t=2
                        )[:, :, off:512]
                        nc.scalar.activation(pv3, sv, Exp, scale=0.125)
                    else:
                        nc.scalar.activation(P[:], stp[:], Exp, scale=0.125)
                    if d >= 0:
                        # triangle mask on the 128-wide diagonal
                        # sub-block of both heads (one DVE op)
                        pv = P[:].rearrange(
                            "p (t i) -> p t i", t=2
                        )[:, :, off : off + 128]
                        cv = cm[:].rearrange("p (t i) -> p t i", t=2)
                        nc.vector.tensor_mul(pv, pv, cv)
                    Ps[jt] = (P, off)

                def emit_y_g(b, p, jt, nj, Ps, yp):
                    P, off = Ps.pop(jt)
                    for e in (0, 1):
                        nc.tensor.matmul(
                            yp[e][:, off:512],
                            vats[jt][:, 2 * p + e, :],
                            P[:, e * 512 + off : (e + 1) * 512],
                            start=(jt == 0),
                            stop=(jt == nj - 1),
                        )

                def emit_norm(b, p, yp):
                    # normalization: denominators (row 64) -> SBUF,
                    # PE K=1 broadcast, DVE reciprocal + scale
                    for e in (0, 1):
                        sden = sdp.tile([1, 512], bf16, tag="sd")
                        nc.vector.tensor_copy(sden[:], yp[e][64:65, :])
                        rb = fillps.tile([64, 512], f32, tag="fill", name="rb")
                        nc.tensor.matmul(
                            rb[:], ones[:], sden[:], start=True, stop=True
                        )
                        rrec = rrp.tile([64, 512], f32, tag="rr")
                        nc.vector.reciprocal_approx_fast(rrec[:], rb[:])
                        nc.vector.tensor_mul(
                            yts[p][e * 64 : (e + 1) * 64,
                                   b * 512 : (b + 1) * 512],
                            yp[e][0:64, :],
                            rrec[:],
                        )

                drained = set()
                done_in_band = {b: 0 for b in range(4)}
                for b, p in order:
                    if b not in drained:
                        drained.add(b)
                        drain_required(b)
                    nj = 4 * b + 4
                    yp = [
                        yps.tile([65, 512], f32, tag="y", name=f"yp{b}{p}{e}")
                        for e in (0, 1)
                    ]
                    Ps = {}
                    for jt in range(nj):
                        emit_st_g(b, p, jt, Ps)
                        if jt % 3 == 2:
                            emit_filler(1)
                        if jt >= LAG:
                            emit_y_g(b, p, jt - LAG, nj, Ps, yp)
                    for jt in range(max(0, nj - LAG), nj):
                        emit_y_g(b, p, jt, nj, Ps, yp)
                        if jt % 2 == 1:
                            emit_filler(1)
                    emit_norm(b, p, yp)
                    if p % 2 == 1:
                        emit_filler(1)

                    # projection of a band becomes available once all four
                    # of its groups are normalized (filler for later bands)
                    done_in_band[b] += 1
                    if done_in_band[b] == 4:
                        for ic in range(4 * b, 4 * b + 4):
                            filler.append(("pj", lambda i=ic: emit_proj_ic(i)))

                while filler:
                    _, fn = filler.popleft()
                    fn()

    nc.compile()
    return nc


def _get_nc():
    if "nc" not in _compiled:
        _compiled["nc"] = _build()
    return _compiled["nc"]


def _prep_inputs(x, Wq, Wk, Wv, Wp, attn_mask):
    import ml_dtypes

    bf16 = ml_dtypes.bfloat16
    x = np.asarray(x, np.float32)
    Wq = np.asarray(Wq, np.float32)
    Wk = np.asarray(Wk, np.float32)
    Wv = np.asarray(Wv, np.float32)
    Wp = np.asarray(Wp, np.float32)
    am = np.asarray(attn_mask)

    # 128x128 lower-triangle causal mask (r <= c), duplicated for the
    # two heads of a pair
    tri = (np.arange(128)[:, None] <= np.arange(128)[None, :]).astype(np.float32)
    cm2 = np.ascontiguousarray(np.tile(tri, (1, 2)).astype(bf16))

    halves = []
    for hh in range(2):
        WqT = Wq[hh * 512 : (hh + 1) * 512, :].T  # (C, 512)
        WkT = Wk[hh * 512 : (hh + 1) * 512, :].T
        WvT = Wv[hh * 512 : (hh + 1) * 512, :].T
        WpT = Wp[:, hh * 512 : (hh + 1) * 512].T  # (512, C)
        # wqk8[kt][:, (2p+w)*128:+128] = W_wT[kt*128:(kt+1)*128, p*128:+128]
        q4 = WqT.reshape(8, 128, 4, 128)  # kt, r, p, c
        k4 = WkT.reshape(8, 128, 4, 128)
        qk = np.stack([q4, k4], axis=3)  # kt, r, p, w, c
        wqk8 = np.ascontiguousarray(
            qk.reshape(8, 128, 1024).astype(bf16)
        )
        wv8 = np.ascontiguousarray(WvT.reshape(8, 128, 512).astype(bf16))
        wp4 = np.ascontiguousarray(WpT.reshape(4, 128, 1024).astype(bf16))
        halves.append((wqk8, wv8, wp4))

    in_maps = []
    for c in range(NCORE):
        b, hh = c // 2, c % 2
        xT = np.ascontiguousarray(x[b].T.astype(bf16)).reshape(8, 128, L)
        padb = (am[b].reshape(16, 128).T != 0).astype(np.float32)
        wqk8, wv8, wp4 = halves[hh]
        in_maps.append(
            {
                "xT8": xT,
                "wqk8": wqk8,
                "wv8": wv8,
                "wp4": wp4,
                "cm2": cm2,
                "one64": np.ones((1, 64), bf16),
                "one8": np.ones((128, 8), bf16),
                "pdb": np.ascontiguousarray(padb),
            }
        )
    return in_maps


def _run(in_maps, trace=False, tmpdir=None):
    from concourse.bass_utils import run_bass_kernel_spmd

    nc = _get_nc()
    if trace:
        _register_ntff_hook()
    return run_bass_kernel_spmd(
        nc, in_maps, list(range(NCORE)), trace=trace, tmpdir=tmpdir
    )


def _register_ntff_hook():
    """The agent image's antenv lacks axon_hooks; register the NTFF
    profiling hook manually so trace=True yields exec_time_ns."""
    import types
    import antenv

    if "antenv.axon_hooks" in sys.modules:
        return
    mod = types.ModuleType("antenv.axon_hooks")
    hook = [None]
    mod.set_axon_ntff_profile_hook = lambda h: hook.__setitem__(0, h)
    mod.get_axon_ntff_profile_hook = lambda: hook[0]
    sys.modules["antenv.axon_hooks"] = mod
    antenv.axon_hooks = mod
    if "/root/.axon_site" not in sys.path:
        sys.path.insert(0, "/root/.axon_site")
    from trn_agent_boot.trn_boot import _ntff_profile_via_ctypes

    mod.set_axon_ntff_profile_hook(
        _ntff_profile_via_ctypes("/opt/axon/libaxon_pjrt.so")
    )


def kernel(x, Wq, Wk, Wv, Wp, attn_mask):
    in_maps = _prep_inputs(x, Wq, Wk, Wv, Wp, attn_mask)
    res = _run(in_maps)
    y = np.empty((B, L, C), np.float32)
    for b in range(B):
        y[b] = res.results[2 * b]["out"] + res.results[2 * b + 1]["out"]
    return y


# revision 19
# speedup vs baseline: 1.1828x; 1.1828x over previous
"""Causal self-attention (B=4, L=2048, C=1024, H=16) on 8 trn2 NeuronCores.

Sharding: hybrid (batch x head) tensor-parallel. Core c handles batch
b = c // 2 and heads [ (c%2)*8, (c%2)*8 + 8 ).  Each core computes a
partial projection output (L, C) for its batch; the host sums the two
partials per batch (the Wp row-sharded all-reduce done host-side).

All matmul inputs are bf16 (PSUM accumulation fp32).  The kernel is a
single software-pipelined PE stream structured so the Tensor engine
never starves (HAM stays at K=8/8):

  - wave0: qT/kT ch0 chains consume x-tiles as their DMAs land
  - attention groups (band it, head-pair p) run ST -> exp -> y with a
    jt-lag; qk waves ch1-3, v chains, projection bands and the softmax
    denominator broadcasts are emitted as independent PE filler so the
    scheduler can hide the exp (ACT) latency under them
  - diagonal score tiles are trimmed: ST/y matmuls only cover the
    causally valid i-range, exp is split for d>=2
  - projection of band b is emitted right after band b's normalization
Per-core dataflow details:
  qT,kT = Wq_h @ x.T      (d, L) layout, head-pair packed (128 rows)
  v_aug = x @ Wv_h.T      (L, d) + ones column for softmax denominators
  ST    = kT_j.T @ qT     two heads at base partitions 0/64 run in
                          separate PE row groups concurrently
  P     = exp(ST/8)       ACT, bf16 out; causal mask via DVE on the
                          128-wide diagonal sub-block only
  yT    = sum_j v_aug_j.T @ P   rows 0..63 out, row 64 = denominator
  yT   *= bcast(1/denom)  K=1 PE broadcast + DVE fast reciprocal
  out   = yT.T @ WpT      partial projection, DMA to DRAM (fp32)
"""
import sys
import os

sys.path.insert(0, "/opt/trn_rl_repo")

import numpy as np

B, L, C, H, HD = 4, 2048, 1024, 16, 64
NCORE = 8

_compiled = {}


def _build():
    import concourse.bass as bass
    import concourse.mybir as mybir
    import concourse.tile as tile
    from concourse import bacc

    dt = mybir.dt
    f32 = dt.float32
    bf16 = dt.bfloat16
    Exp = mybir.ActivationFunctionType.Exp
    PSUM = bass.MemorySpace.PSUM

    nc = bacc.Bacc("TRN2", target_bir_lowering=False, debug=False, num_devices=NCORE)

    xT8 = nc.dram_tensor("xT8", [8, 128, L], bf16, kind="ExternalInput")
    wqk8 = nc.dram_tensor("wqk8", [8, 128, 1024], bf16, kind="ExternalInput")
    wv8 = nc.dram_tensor("wv8", [8, 128, 512], bf16, kind="ExternalInput")
    wp4 = nc.dram_tensor("wp4", [4, 128, 1024], bf16, kind="ExternalInput")
    cm2 = nc.dram_tensor("cm2", [128, 256], bf16, kind="ExternalInput")
    pdb = nc.dram_tensor("pdb", [128, 16], f32, kind="ExternalInput")
    one64 = nc.dram_tensor("one64", [1, 64], bf16, kind="ExternalInput")
    one8 = nc.dram_tensor("one8", [128, 8], bf16, kind="ExternalInput")
    out = nc.dram_tensor("out", [L, C], f32, kind="ExternalOutput")

    with tile.TileContext(nc) as tc:
        with (
            tc.tile_pool(name="persist", bufs=1) as persist,
            tc.tile_pool(name="xw", bufs=1) as xw,
            tc.tile_pool(name="qkT", bufs=8) as qkp,
            tc.tile_pool(name="vaug", bufs=16) as vaugp,
            tc.tile_pool(name="yT", bufs=4) as ytp,
            tc.tile_pool(name="P", bufs=6) as Pp,
            tc.tile_pool(name="sd", bufs=4) as sdp,
            tc.tile_pool(name="rr", bufs=4) as rrp,
            tc.tile_pool(name="osb", bufs=2) as osbp,
        ):
            # small constants first (cheap DMAs)
            pb = persist.tile([128, 16], f32, tag="pb")
            nc.sync.dma_start(pb[:], pdb[:])
            ones = persist.tile([1, 64], bf16, tag="ones")
            nc.sync.dma_start(ones[:], one64[:])
            ones8 = persist.tile([128, 8], bf16, tag="ones8")
            nc.sync.dma_start(ones8[:], one8[:])
            cm = persist.tile([128, 256], bf16, tag="cm")
            nc.sync.dma_start(cm[:], cm2[:])

            # weights + x interleaved so wave0 can start on the first tile
            # input load split across the two HWDGE queues (sync + scalar)
            # so the transfers run in parallel and wave0 starts sooner
            wqkts, xts = [], []
            for kt in range(8):
                w = xw.tile([128, 1024], bf16, tag="wqk", name=f"wqk{kt}", bufs=8)
                (nc.sync if kt % 2 else nc.scalar).dma_start(w[:], wqk8[kt])
                wqkts.append(w)
                t = xw.tile([128, L], bf16, tag="x", name=f"x{kt}", bufs=8)
                # wave0 only reads cols 0:512 — land that head first so the
                # first chains start right after the runtime preamble
                (nc.scalar if kt % 2 else nc.sync).dma_start(
                    t[:, 0:512], xT8[kt][:, 0:512]
                )
                xts.append(t)
            for kt in range(8):
                (nc.scalar if kt % 2 else nc.sync).dma_start(
                    xts[kt][:, 512:L], xT8[kt][:, 512:L]
                )
            wvts = []
            for kt in range(8):
                t = xw.tile([128, 512], bf16, tag="wv", name=f"wv{kt}", bufs=8)
                (nc.scalar if kt % 2 else nc.sync).dma_start(t[:], wv8[kt])
                wvts.append(t)
            wpts = []
            for kt4 in range(4):
                t = xw.tile([128, 1024], bf16, tag="wp", name=f"wp{kt4}", bufs=4)
                (nc.scalar if kt4 % 2 else nc.sync).dma_start(t[:], wp4[kt4])
                wpts.append(t)

            # warm the exp activation table while wave0 runs
            scratch = persist.tile([128, 16], f32, tag="scr")
            nc.scalar.activation(scratch[:], pb[:], Exp, scale=0.0)

            qts = [qkp.tile([128, L], bf16, tag="qk", name=f"q{p}") for p in range(4)]
            kts = [qkp.tile([128, L], bf16, tag="qk", name=f"k{p}") for p in range(4)]
            vats = [
                vaugp.tile([128, 8, 65], bf16, tag="va", name=f"va{it}")
                for it in range(16)
            ]
            yts = [ytp.tile([128, L], bf16, tag="yt", name=f"yt{p}") for p in range(4)]

            # ---- wave0: qk chains for ch 0 (8 chains, one PSUM bank each);
            # kt-outer issue so chains advance as x tiles arrive ----
            with tc.tile_pool(name="mmA", bufs=8, space=PSUM) as mmA:
                ps0 = [
                    mmA.tile([128, 512], f32, tag="mm", name=f"w0_{c8}")
                    for c8 in range(8)
                ]
                for kt in range(8):
                    for c8 in range(8):
                        nc.tensor.matmul(
                            ps0[c8][:],
                            wqkts[kt][:, c8 * 128 : (c8 + 1) * 128],
                            xts[kt][:, 0:512],
                            start=(kt == 0),
                            stop=(kt == 7),
                        )
                for c8 in range(8):
                    p, w = c8 // 2, c8 % 2
                    dst = qts[p] if w == 0 else kts[p]
                    nc.vector.tensor_copy(dst[:, 0:512], ps0[c8][:])

            # ---- attention + filler stream ----
            with (
                tc.tile_pool(name="stps", bufs=2, space=PSUM) as stps,
                tc.tile_pool(name="fillps", bufs=2, space=PSUM) as fillps,
                tc.tile_pool(name="yps", bufs=2, space=PSUM) as yps,
            ):
                def emit_qk_chain(p, w, ch):
                    ps = fillps.tile([128, 512], f32, tag="fill", name="qkc")
                    c8 = 2 * p + w
                    for kt in range(8):
                        nc.tensor.matmul(
                            ps[:],
                            wqkts[kt][:, c8 * 128 : (c8 + 1) * 128],
                            xts[kt][:, ch * 512 : (ch + 1) * 512],
                            start=(kt == 0),
                            stop=(kt == 7),
                        )
                    dst = qts[p] if w == 0 else kts[p]
                    nc.vector.tensor_copy(
                        dst[:, ch * 512 : (ch + 1) * 512], ps[:]
                    )

                def emit_v_chain(it):
                    ps = fillps.tile([128, 512], f32, tag="fill", name="vc")
                    for kt in range(8):
                        nc.tensor.matmul(
                            ps[:],
                            xts[kt][:, it * 128 : (it + 1) * 128],
                            wvts[kt][:],
                            start=(kt == 0),
                            stop=(kt == 7),
                        )
                    va = vats[it]
                    nc.vector.tensor_scalar_mul(
                        va[:, :, 0:64],
                        ps[:].rearrange("p (h e) -> p h e", e=64),
                        pb[:, it : it + 1],
                    )
                    nc.vector.tensor_scalar_mul(
                        va[:, :, 64:65],
                        ones8[:].rearrange("p (h o) -> p h o", o=1),
                        pb[:, it : it + 1],
                    )

                def emit_proj_ic(ic):
                    osb = osbp.tile([128, 1024], f32, tag="o", name="osb")
                    for ch in range(2):
                        ps = fillps.tile([128, 512], f32, tag="fill", name="pj")
                        for kt4 in range(4):
                            nc.tensor.matmul(
                                ps[:],
                                yts[kt4][:, ic * 128 : (ic + 1) * 128],
                                wpts[kt4][:, ch * 512 : (ch + 1) * 512],
                                start=(kt4 == 0),
                                stop=(kt4 == 3),
                            )
                        nc.vector.tensor_copy(
                            osb[:, ch * 512 : (ch + 1) * 512], ps[:]
                        )
                    nc.sync.dma_start(out[ic * 128 : (ic + 1) * 128, :], osb[:])

                # filler queue: list of closures, drained between attention
                # tiles; order = priority hint for the scheduler
                from collections import deque

                filler = deque()
                for ch in range(1, 4):
                    for c8 in range(8):
                        filler.append((f"w{ch}", lambda p=c8 // 2, w=c8 % 2,
                                       c=ch: emit_qk_chain(p, w, c)))
                for it in range(16):
                    filler.append((f"v{it}", lambda i=it: emit_v_chain(i)))

                def drain_required(b):
                    """Everything band b's STs/y's need must be emitted."""
                    need_w = {f"w{c}" for c in range(1, b + 1)}
                    need_v = {f"v{i}" for i in range(4 * b + 4)}
                    need = need_w | need_v
                    keep = deque()
                    while filler:
                        tag, fn = filler.popleft()
                        if tag in need:
                            fn()
                        else:
                            keep.append((tag, fn))
                    filler.extend(keep)

                def emit_filler(n):
                    for _ in range(n):
                        if not filler:
                            return
                        _, fn = filler.popleft()
                        fn()

                LAG = 4

                # bands 2 and 3 are interleaved group-wise: band 3 is
                # ACT-heavy (longest exp rows) while band 2 still has qk
                # wave 3 + v chains as PE filler — mixing them keeps both
                # engines fed through the back half of the kernel
                order = [(0, 0), (0, 1), (0, 2), (0, 3),
                         (1, 0), (1, 1), (1, 2), (1, 3),
                         (2, 0), (3, 0), (2, 1), (3, 1),
                         (2, 2), (3, 2), (2, 3), (3, 3)]
                def emit_st_g(b, p, jt, Ps, ptag="P", pbufs=None):
                    d = jt - 4 * b  # diagonal alignment, >=0 on diag
                    off = 128 * d if d > 0 else 0
                    qT, kT = qts[p], kts[p]
                    stp = stps.tile([128, 1024], f32, tag="st", name="stp")
                    for e in (0, 1):
                        ro = e * 64
                        nc.tensor.matmul(
                            stp[:, e * 512 + off : (e + 1) * 512],
                            kT[ro : ro + 64, jt * 128 : (jt + 1) * 128],
                            qT[ro : ro + 64, b * 512 + off : (b + 1) * 512],
                            start=True,
                            stop=True,
                        )
                    P = Pp.tile([128, 1024], bf16, tag=ptag, name="P",
                                bufs=pbufs)
                    if d >= 1:
                        # exp only the causally valid column range of
                        # both heads in one strided instruction (also
                        # avoids reading never-written PSUM)
                        sv = stp[:].rearrange(
                            "p (t i) -> p t i", t=2
                        )[:, :, off:512]
                        pv3 = P[:].rearrange(
                            "p (t i) -> p t i", t=2
                        )[:, :, off:512]
                        nc.scalar.activation(pv3, sv, Exp, scale=0.125)
                    else:
                        nc.scalar.activation(P[:], stp[:], Exp, scale=0.125)
                    if d >= 0:
                        # triangle mask on the 128-wide diagonal
                        # sub-block of both heads (one DVE op)
                        pv = P[:].rearrange(
                            "p (t i) -> p t i", t=2
                        )[:, :, off : off + 128]
                        cv = cm[:].rearrange("p (t i) -> p t i", t=2)
                        nc.vector.tensor_mul(pv, pv, cv)
                    Ps[jt] = (P, off)

                def emit_y_g(b, p, jt, nj, Ps, yp):
                    P, off = Ps.pop(jt)
                    for e in (0, 1):
                        nc.tensor.matmul(
                            yp[e][:, off:512],
                            vats[jt][:, 2 * p + e, :],
                            P[:, e * 512 + off : (e + 1) * 512],
                            start=(jt == 0),
                            stop=(jt == nj - 1),
                        )

                def emit_norm(b, p, yp):
                    # normalization: denominators (row 64) -> SBUF,
                    # PE K=1 broadcast, DVE reciprocal + scale
                    for e in (0, 1):
                        sden = sdp.tile([1, 512], bf16, tag="sd")
                        nc.vector.tensor_copy(sden[:], yp[e][64:65, :])
                        rb = fillps.tile([64, 512], f32, tag="fill", name="rb")
                        nc.tensor.matmul(
                            rb[:], ones[:], sden[:], start=True, stop=True
                        )
                        rrec = rrp.tile([64, 512], f32, tag="rr")
                        nc.vector.reciprocal_approx_fast(rrec[:], rb[:])
                        nc.vector.tensor_mul(
                            yts[p][e * 64 : (e + 1) * 64,
                                   b * 512 : (b + 1) * 512],
                            yp[e][0:64, :],
                            rrec[:],
                        )

                drained = set()
                done_in_band = {b: 0 for b in range(4)}
                for b, p in order:
                    if b not in drained:
                        drained.add(b)
                        drain_required(b)
                    nj = 4 * b + 4
                    yp = [
                        yps.tile([65, 512], f32, tag="y", name=f"yp{b}{p}{e}")
                        for e in (0, 1)
                    ]
                    Ps = {}
                    for jt in range(nj):
                        emit_st_g(b, p, jt, Ps)
                        if jt % 3 == 2:
                            emit_filler(1)
                        if jt >= LAG:
                            emit_y_g(b, p, jt - LAG, nj, Ps, yp)
                    for jt in range(max(0, nj - LAG), nj):
                        emit_y_g(b, p, jt, nj, Ps, yp)
                    emit_norm(b, p, yp)
                    if p % 2 == 1:
                        emit_filler(1)

                    # projection of a band becomes available once all four
                    # of its groups are normalized (filler for later bands)
                    done_in_band[b] += 1
                    if done_in_band[b] == 4:
                        for ic in range(4 * b, 4 * b + 4):
                            filler.append(("pj", lambda i=ic: emit_proj_ic(i)))

                while filler:
                    _, fn = filler.popleft()
                    fn()

    nc.compile()
    return nc


def _get_nc():
    if "nc" not in _compiled:
        _compiled["nc"] = _build()
    return _compiled["nc"]


def _prep_inputs(x, Wq, Wk, Wv, Wp, attn_mask):
    import ml_dtypes

    bf16 = ml_dtypes.bfloat16
    x = np.asarray(x, np.float32)
    Wq = np.asarray(Wq, np.float32)
    Wk = np.asarray(Wk, np.float32)
    Wv = np.asarray(Wv, np.float32)
    Wp = np.asarray(Wp, np.float32)
    am = np.asarray(attn_mask)

    # 128x128 lower-triangle causal mask (r <= c), duplicated for the
    # two heads of a pair
    tri = (np.arange(128)[:, None] <= np.arange(128)[None, :]).astype(np.float32)
    cm2 = np.ascontiguousarray(np.tile(tri, (1, 2)).astype(bf16))

    halves = []
    for hh in range(2):
        WqT = Wq[hh * 512 : (hh + 1) * 512, :].T  # (C, 512)
        WkT = Wk[hh * 512 : (hh + 1) * 512, :].T
        WvT = Wv[hh * 512 : (hh + 1) * 512, :].T
        WpT = Wp[:, hh * 512 : (hh + 1) * 512].T  # (512, C)
        # wqk8[kt][:, (2p+w)*128:+128] = W_wT[kt*128:(kt+1)*128, p*128:+128]
        q4 = WqT.reshape(8, 128, 4, 128)  # kt, r, p, c
        k4 = WkT.reshape(8, 128, 4, 128)
        qk = np.stack([q4, k4], axis=3)  # kt, r, p, w, c
        wqk8 = np.ascontiguousarray(
            qk.reshape(8, 128, 1024).astype(bf16)
        )
        wv8 = np.ascontiguousarray(WvT.reshape(8, 128, 512).astype(bf16))
        wp4 = np.ascontiguousarray(WpT.reshape(4, 128, 1024).astype(bf16))
        halves.append((wqk8, wv8, wp4))

    in_maps = []
    for c in range(NCORE):
        b, hh = c // 2, c % 2
        xT = np.ascontiguousarray(x[b].T.astype(bf16)).reshape(8, 128, L)
        padb = (am[b].reshape(16, 128).T != 0).astype(np.float32)
        wqk8, wv8, wp4 = halves[hh]
        in_maps.append(
            {
                "xT8": xT,
                "wqk8": wqk8,
                "wv8": wv8,
                "wp4": wp4,
                "cm2": cm2,
                "one64": np.ones((1, 64), bf16),
                "one8": np.ones((128, 8), bf16),
                "pdb": np.ascontiguousarray(padb),
            }
        )
    return in_maps


def _run(in_maps, trace=False, tmpdir=None):
    from concourse.bass_utils import run_bass_kernel_spmd

    nc = _get_nc()
    if trace:
        _register_ntff_hook()
    return run_bass_kernel_spmd(
        nc, in_maps, list(range(NCORE)), trace=trace, tmpdir=tmpdir
    )


def _register_ntff_hook():
    """The agent image's antenv lacks axon_hooks; register the NTFF
    profiling hook manually so trace=True yields exec_time_ns."""
    import types
    import antenv

    if "antenv.axon_hooks" in sys.modules:
        return
    mod = types.ModuleType("antenv.axon_hooks")
    hook = [None]
    mod.set_axon_ntff_profile_hook = lambda h: hook.__setitem__(0, h)
    mod.get_axon_ntff_profile_hook = lambda: hook[0]
    sys.modules["antenv.axon_hooks"] = mod
    antenv.axon_hooks = mod
    if "/root/.axon_site" not in sys.path:
        sys.path.insert(0, "/root/.axon_site")
    from trn_agent_boot.trn_boot import _ntff_profile_via_ctypes

    mod.set_axon_ntff_profile_hook(
        _ntff_profile_via_ctypes("/opt/axon/libaxon_pjrt.so")
    )


def kernel(x, Wq, Wk, Wv, Wp, attn_mask):
    in_maps = _prep_inputs(x, Wq, Wk, Wv, Wp, attn_mask)
    res = _run(in_maps)
    y = np.empty((B, L, C), np.float32)
    for b in range(B):
        y[b] = res.results[2 * b]["out"] + res.results[2 * b + 1]["out"]
    return y
